# revision 9
# baseline (speedup 1.0000x reference)
"""Trainium2 Bass kernel for a 2-layer GAT (nn_GAT_70909910057105).

Strategy (8 NeuronCores, SPMD):
  - Core k owns target nodes [128k, 128k+128). Edges are bucketed by trg//128
    on the host (integer-only preprocessing), then sub-bucketed by src//256 so
    edge-feature rows can be gathered with int16 indices.
  - A DRAM "node table" holds per-node rows [h (1024 f32) | a_src (16) |
    a_tgt (16) | pad (32)].  Per-edge source rows are fetched with dma_gather;
    per-edge target alphas with a second (sub-row) dma_gather.
  - segment_sum becomes a PSUM-accumulated matmul with host-built one-hot
    masks: out[t, :] += mask_chunk.T @ (exp ⊙ h_src_chunk).
  - Edge-feature projection pe = (e_feats @ We.T).sum_per_head * a_e collapses
    to e_feats @ wesum (wesum computed on device), done once for both layers.
  - One AllGather per layer rebuilds the replicated node table from per-core
    node rows.
"""
import sys

for _p in ("/opt/trn_rl_repo", "/root/.axon_site/_ro/trn_rl_repo"):
    if _p not in sys.path:
        sys.path.insert(0, _p)

import numpy as np
import concourse.bass as bass
import concourse.bacc as bacc
import concourse.tile as tile
from concourse import mybir
from concourse.bass_utils import run_bass_kernel_spmd
from concourse.masks import make_identity

F32 = mybir.dt.float32
I16 = mybir.dt.int16

N, B, C, H, D = 1024, 4, 256, 4, 64
E = 32768
NC = 8
TPC = N // NC          # target nodes per core = 128
ROW = 1088             # node-table row: 1024 h | 16 a_src | 16 a_tgt | 32 pad
NB_LOCAL = TPC * B     # 512 local (node, batch) rows
Q = 4                  # src quarters (for int16 edge-feature indexing)
QROWS = (N // Q) * TPC  # 32768 rows per edge-feature shard quarter


# --------------------------------------------------------------------------
# host-side preprocessing (integer / layout ops only)
# --------------------------------------------------------------------------

def _pack_idx(vals: np.ndarray) -> np.ndarray:
    """Pack flat int indices for dma_gather: slot i at [i % 16, i // 16],
    replicated across the 8 gpsimd core partition groups."""
    n = vals.shape[0]
    assert n % 16 == 0
    blk = vals.astype(np.int16).reshape(n // 16, 16).T
    return np.ascontiguousarray(np.tile(blk, (8, 1)))


def _prep(x, edge_features, src_idx, trg_idx,
          Wn1, We1, a_src1, a_tgt1, a_edge1,
          Wn2, We2, a_src2, a_tgt2, a_edge2):
    src = np.asarray(src_idx).astype(np.int64)
    trg = np.asarray(trg_idx).astype(np.int64)
    x = np.asarray(x, dtype=np.float32)
    ef = np.asarray(edge_features, dtype=np.float32)

    # bucket edges: core = trg // TPC, sub-bucket = src // 256
    per_core = []
    bmax = 0
    for k in range(NC):
        eids = np.nonzero((trg // TPC) == k)[0]
        bks = [eids[(src[eids] // (N // Q)) == q] for q in range(Q)]
        per_core.append(bks)
        bmax = max(bmax, max(len(b) for b in bks))
    B_pad = ((bmax + 127) // 128) * 128
    E_pad = Q * B_pad
    n_chunks = E_pad // 128

    xf = x.reshape(N * B, C)          # (n, b) rows, b inner
    xT = np.ascontiguousarray(xf.T)   # [C, N*B]

    def sb3(w, inner):  # [256, inner] -> sbuf layout [128, 2, inner]
        return np.ascontiguousarray(w.reshape(2, 128, inner).transpose(1, 0, 2))

    def hsel(a_e):
        m = np.zeros((C, H), np.float32)
        for h in range(H):
            m[h * D:(h + 1) * D, h] = np.float32(a_e[h])
        return sb3(m, H)

    def ablk(a_s, a_t):
        m = np.zeros((C, 2 * H), np.float32)
        for h in range(H):
            m[h * D:(h + 1) * D, h] = np.asarray(a_s)[h]
            m[h * D:(h + 1) * D, H + h] = np.asarray(a_t)[h]
        return sb3(m, 2 * H)

    common = {
        "wn1hd": sb3(np.asarray(Wn1, np.float32), C),
        "wn2hd": sb3(np.asarray(Wn2, np.float32), C),
        "wn1cols": sb3(np.ascontiguousarray(np.asarray(Wn1, np.float32).T), C),
        "wn2cols": sb3(np.ascontiguousarray(np.asarray(Wn2, np.float32).T), C),
        "we1hd": sb3(np.asarray(We1, np.float32), C),
        "we2hd": sb3(np.asarray(We2, np.float32), C),
        "hsel1": hsel(np.asarray(a_edge1)),
        "hsel2": hsel(np.asarray(a_edge2)),
        "ablk1": ablk(a_src1, a_tgt1),
        "ablk2": ablk(a_src2, a_tgt2),
    }

    in_maps = []
    for k in range(NC):
        src_s = np.zeros(E_pad, np.int64)
        trg_s = np.zeros(E_pad, np.int64)
        efi_s = np.zeros(E_pad, np.int64)
        mask = np.zeros((128, E_pad), np.float32)
        for q in range(Q):
            ids = per_core[k][q]
            s0 = q * B_pad
            sl = slice(s0, s0 + len(ids))
            src_s[sl] = src[ids]
            trg_s[sl] = trg[ids]
            tl = trg[ids] - k * TPC
            efi_s[sl] = (src[ids] - q * (N // Q)) * TPC + tl
            for i, (sslot, t) in enumerate(zip(range(s0, s0 + len(ids)), tl)):
                mask[sslot % 128, (sslot // 128) * 128 + t] = 1.0
        shard = np.ascontiguousarray(
            ef[:, k * TPC:(k + 1) * TPC, :]).reshape(N * TPC, C)
        m = dict(common)
        m.update({
            "ef": shard,
            "xT": np.ascontiguousarray(
                xT[:, k * NB_LOCAL:(k + 1) * NB_LOCAL]
            ).reshape(2, 128, NB_LOCAL).transpose(1, 0, 2).copy(),
            "isrc": _pack_idx(src_s),
            "itrg": _pack_idx(trg_s),
            "ief": _pack_idx(efi_s),
            "mask": mask,
        })
        in_maps.append(m)
    return in_maps, B_pad, E_pad, n_chunks


# --------------------------------------------------------------------------
# device program
# --------------------------------------------------------------------------

def _build(B_pad: int, debug: bool = False, stop_after: str = 'full'):
    E_pad = Q * B_pad
    n_chunks = E_pad // 128
    n_super = E_pad // 512
    nc = bacc.Bacc("TRN2", target_bir_lowering=False, debug=False,
                   num_devices=NC)

    ef_in = nc.dram_tensor("ef", [Q * QROWS, C], F32, kind="ExternalInput")
    xT_in = nc.dram_tensor("xT", [128, 2, NB_LOCAL], F32, kind="ExternalInput")
    isrc_in = nc.dram_tensor("isrc", [128, E_pad // 16], I16, kind="ExternalInput")
    itrg_in = nc.dram_tensor("itrg", [128, E_pad // 16], I16, kind="ExternalInput")
    ief_in = nc.dram_tensor("ief", [128, E_pad // 16], I16, kind="ExternalInput")
    mask_in = nc.dram_tensor("mask", [128, E_pad], F32, kind="ExternalInput")
    w_in = {
        nm: nc.dram_tensor(nm, [128, 2, inner], F32, kind="ExternalInput")
        for nm, inner in [
            ("wn1hd", C), ("wn2hd", C), ("wn1cols", C), ("wn2cols", C),
            ("we1hd", C), ("we2hd", C),
            ("hsel1", H), ("hsel2", H), ("ablk1", 2 * H), ("ablk2", 2 * H),
        ]
    }
    y_out = nc.dram_tensor("y", [128, B * C], F32, kind="ExternalOutput")
    dbg = {}
    if debug:
        for nm, shape in [("dbg_x1", [128, B * C]), ("dbg_pe", [128, n_chunks, 8]),
                          ("dbg_den", [128, 16]), ("dbg_tbl", [N, ROW])]:
            dbg[nm] = nc.dram_tensor(nm, shape, F32, kind="ExternalOutput")

    from contextlib import ExitStack
    with tile.TileContext(nc) as tc:
        with ExitStack() as ctx:
            const = ctx.enter_context(tc.tile_pool(name="const", bufs=1))
            sb = ctx.enter_context(tc.tile_pool(name="sb", bufs=1))
            small = ctx.enter_context(tc.tile_pool(name="small", bufs=3))
            gpool = ctx.enter_context(tc.tile_pool(name="gpool", bufs=3))
            apool = ctx.enter_context(tc.tile_pool(name="apool", bufs=3))
            efpool = ctx.enter_context(tc.tile_pool(name="efpool", bufs=2))
            ps_small = ctx.enter_context(
                tc.tile_pool(name="ps_small", bufs=2, space="PSUM"))
            ps_t = ctx.enter_context(
                tc.tile_pool(name="ps_t", bufs=2, space="PSUM"))
            ps_out = ctx.enter_context(
                tc.tile_pool(name="ps_out", bufs=1, space="PSUM"))
            ps_den = ctx.enter_context(
                tc.tile_pool(name="ps_den", bufs=1, space="PSUM"))
            dram = ctx.enter_context(tc.tile_pool(name="dram", bufs=1, space="DRAM"))

            ident = const.tile([128, 128], F32)
            make_identity(nc, ident[:])
            zpad = const.tile([128, 2 * H], F32)
            nc.vector.memset(zpad[:], 0.0)

            # ---- load constants / inputs to SBUF
            w_sb = {}
            for nm, t in w_in.items():
                inner = t.shape[2]
                w_sb[nm] = const.tile([128, 2, inner], F32, name=f"w_{nm}",
                                      tag=f"w_{nm}")
                nc.sync.dma_start(out=w_sb[nm][:], in_=t[:])
            xT_sb = const.tile([128, 2, NB_LOCAL], F32)
            nc.sync.dma_start(out=xT_sb[:], in_=xT_in[:])
            isrc_t = const.tile([128, E_pad // 16], I16)
            nc.sync.dma_start(out=isrc_t[:], in_=isrc_in[:])
            itrg_t = const.tile([128, E_pad // 16], I16)
            nc.sync.dma_start(out=itrg_t[:], in_=itrg_in[:])
            ief_t = const.tile([128, E_pad // 16], I16)
            nc.sync.dma_start(out=ief_t[:], in_=ief_in[:])
            mask_sb = const.tile([128, E_pad], F32)
            nc.sync.dma_start(out=mask_sb[:], in_=mask_in[:])

            # ---- wesum[c, (layer, h)] and A[c, (s/t, h)] prep matmuls
            wesum_sb = const.tile([128, 2, 2 * H], F32)   # [c-part, c-half, 8]
            a1_sb = const.tile([128, 2, 2 * H], F32)
            a2_sb = const.tile([128, 2, 2 * H], F32)
            for ct in range(2):
                pw = ps_small.tile([128, 2 * H], F32, space="PSUM", tag="ps", name="pw")
                for lj, (wehd, hs) in enumerate(
                        [("we1hd", "hsel1"), ("we2hd", "hsel2")]):
                    for kh in range(2):
                        nc.tensor.matmul(
                            out=pw[:, lj * H:(lj + 1) * H],
                            lhsT=w_sb[wehd][:, kh, ct * 128:(ct + 1) * 128],
                            rhs=w_sb[hs][:, kh, :],
                            start=(kh == 0), stop=(kh == 1))
                nc.scalar.copy(out=wesum_sb[:, ct, :], in_=pw[:])
                for dst, wnhd, ab in [(a1_sb, "wn1hd", "ablk1"),
                                      (a2_sb, "wn2hd", "ablk2")]:
                    pa = ps_small.tile([128, 2 * H], F32, space="PSUM", tag="ps", name="pa")
                    for kh in range(2):
                        nc.tensor.matmul(
                            out=pa[:],
                            lhsT=w_sb[wnhd][:, kh, ct * 128:(ct + 1) * 128],
                            rhs=w_sb[ab][:, kh, :],
                            start=(kh == 0), stop=(kh == 1))
                    nc.scalar.copy(out=dst[:, ct, :], in_=pa[:])

            # ---- phase A: pe[e, (layer, h)] for all edge slots
            pe_sb = sb.tile([128, n_chunks, 2 * H], F32)
            for q in range(Q):
                eft = efpool.tile([128, B_pad // 128, C], F32)
                nc.gpsimd.dma_gather(
                    out_ap=eft[:],
                    in_ap=ef_in[q * QROWS:(q + 1) * QROWS, :],
                    idxs_ap=ief_t[:, q * (B_pad // 16):(q + 1) * (B_pad // 16)],
                    num_idxs=B_pad, num_idxs_reg=B_pad, elem_size=C,
                    single_packet=False)
                for jc in range(B_pad // 128):
                    c = q * (B_pad // 128) + jc
                    eT = small.tile([128, 2, 128], F32, tag="eT")
                    for ch in range(2):
                        pt = ps_t.tile([128, 128], F32, space="PSUM", tag="pt", name="pt")
                        nc.tensor.transpose(
                            out=pt[:], in_=eft[:, jc, ch * 128:(ch + 1) * 128],
                            identity=ident[:])
                        nc.scalar.copy(out=eT[:, ch, :], in_=pt[:])
                    pp = ps_small.tile([128, 2 * H], F32, space="PSUM", tag="ps", name="pp")
                    for ch in range(2):
                        nc.tensor.matmul(
                            out=pp[:], lhsT=eT[:, ch, :],
                            rhs=wesum_sb[:, ch, :],
                            start=(ch == 0), stop=(ch == 1))
                    nc.scalar.copy(out=pe_sb[:, c, :], in_=pp[:])
            if debug:
                nc.sync.dma_start(out=dbg["dbg_pe"][:], in_=pe_sb[:])
            if stop_after == 'A':
                dummy = sb.tile([128, B * C], F32)
                nc.vector.memset(dummy[:], 0.0)
                nc.vector.tensor_scalar_add(dummy[:, 0:8], pe_sb[:, 0, :], 0.0)
                nc.sync.dma_start(out=y_out[:], in_=dummy[:])

            # ---- local node-table build + AllGather
            def build_table(lhsT_sb, wncols, a_sb, tag):
                ag_in = dram.tile([TPC, ROW], F32, tag=f"agin{tag}")
                table = dram.tile([N, ROW], F32, addr_space="Shared",
                                  tag=f"tbl{tag}")
                for t in range(4):
                    ph = ps_small.tile([128, C], F32, space="PSUM", tag="ps", name="ph")
                    pa = ps_small.tile([128, 2 * H], F32, space="PSUM", tag="ps", name="pa")
                    for ch in range(2):
                        lhsT = lhsT_sb[:, ch, t * 128:(t + 1) * 128]
                        nc.tensor.matmul(out=ph[:], lhsT=lhsT,
                                         rhs=wncols[:, ch, :],
                                         start=(ch == 0), stop=(ch == 1))
                        nc.tensor.matmul(out=pa[:], lhsT=lhsT,
                                         rhs=a_sb[:, ch, :],
                                         start=(ch == 0), stop=(ch == 1))
                    sh = small.tile([128, C], F32, tag="sh")
                    sa = small.tile([128, 2 * H], F32, tag="sa")
                    nc.scalar.copy(out=sh[:], in_=ph[:])
                    nc.scalar.copy(out=sa[:], in_=pa[:])
                    rows = slice(t * 32, (t + 1) * 32)
                    nc.sync.dma_start(
                        out=ag_in[rows, 0:B * C].rearrange(
                            "n (b o) -> n b o", b=B),
                        in_=sh[:])
                    nc.sync.dma_start(
                        out=ag_in[rows, B * C:B * C + B * H].rearrange(
                            "n (b h) -> n b h", b=B),
                        in_=sa[:, 0:H])
                    nc.sync.dma_start(
                        out=ag_in[rows, B * C + B * H:B * C + 2 * B * H].rearrange(
                            "n (b h) -> n b h", b=B),
                        in_=sa[:, H:2 * H])
                    nc.sync.dma_start(
                        out=ag_in[rows, B * C + 2 * B * H:ROW].rearrange(
                            "n (b z) -> n b z", b=B),
                        in_=zpad[:])
                nc.gpsimd.collective_compute(
                    "AllGather", mybir.AluOpType.bypass,
                    replica_groups=[list(range(NC))],
                    ins=[ag_in.opt()], outs=[table.opt()])
                return table

            if stop_after != 'A':
              table1 = build_table(xT_sb, w_sb["wn1cols"], a1_sb, 1)
              if stop_after == 'B':
                dummy = sb.tile([128, B * C], F32)
                nc.sync.dma_start(out=dummy[:], in_=table1[0:128, 0:B * C])
                nc.sync.dma_start(out=y_out[:], in_=dummy[:])

            # ---- edge loop for one layer
            def edge_loop(table, layer):
                out_p = ps_out.tile([128, B * C], F32, space="PSUM", tag="out")
                den_p = ps_den.tile([128, B * H], F32, space="PSUM", tag="den")
                for s in range(n_super):
                    G = gpool.tile([128, 4, ROW], F32, tag="G")
                    nc.gpsimd.dma_gather(
                        out_ap=G[:], in_ap=table[:],
                        idxs_ap=isrc_t[:, s * 32:(s + 1) * 32],
                        num_idxs=512, num_idxs_reg=512, elem_size=ROW)
                    T = apool.tile([128, 4, 64], F32, tag="T")
                    nc.gpsimd.dma_gather(
                        out_ap=T[:], in_ap=table[:, B * C:B * C + 64],
                        idxs_ap=itrg_t[:, s * 32:(s + 1) * 32],
                        num_idxs=512, num_idxs_reg=512, elem_size=64,
                        elem_step=ROW)
                    for j in range(4):
                        c = s * 4 + j
                        s_sb = small.tile([128, B * H], F32, tag="s")
                        t_sb = small.tile([128, B * H], F32, tag="t")
                        e_sb = small.tile([128, B * H], F32, tag="e")
                        nc.vector.tensor_tensor(
                            out=s_sb[:], in0=G[:, j, B * C:B * C + B * H],
                            in1=T[:, j, 16:16 + B * H],
                            op=mybir.AluOpType.add)
                        nc.vector.tensor_tensor(
                            out=s_sb[:].rearrange("p (b h) -> p b h", b=B),
                            in0=s_sb[:].rearrange("p (b h) -> p b h", b=B),
                            in1=pe_sb[:, c:c + 1, layer * H:(layer + 1) * H]
                                .to_broadcast([128, B, H]),
                            op=mybir.AluOpType.add)
                        nc.scalar.mul(out=t_sb[:], in_=s_sb[:], mul=0.2)
                        nc.vector.tensor_tensor(
                            out=s_sb[:], in0=s_sb[:], in1=t_sb[:],
                            op=mybir.AluOpType.max)
                        nc.scalar.activation(
                            out=e_sb[:], in_=s_sb[:],
                            func=mybir.ActivationFunctionType.Exp)
                        nc.vector.tensor_tensor(
                            out=G[:, j, 0:B * C].rearrange(
                                "p (x d) -> p x d", d=D),
                            in0=G[:, j, 0:B * C].rearrange(
                                "p (x d) -> p x d", d=D),
                            in1=e_sb[:].rearrange("p (x u) -> p x u", u=1)
                                .to_broadcast([128, B * H, D]),
                            op=mybir.AluOpType.mult)
                        mk = mask_sb[:, c * 128:(c + 1) * 128]
                        first, last = (c == 0), (c == n_chunks - 1)
                        nc.tensor.matmul(out=out_p[:, 0:512], lhsT=mk,
                                         rhs=G[:, j, 0:512],
                                         start=first, stop=last)
                        nc.tensor.matmul(out=out_p[:, 512:1024], lhsT=mk,
                                         rhs=G[:, j, 512:1024],
                                         start=first, stop=last)
                        nc.tensor.matmul(out=den_p[:], lhsT=mk, rhs=e_sb[:],
                                         start=first, stop=last)
                # finalize: x = out / (den + eps)
                dsb = small.tile([128, B * H], F32, tag="d")
                nc.vector.tensor_scalar_add(dsb[:], den_p[:], 1e-16)
                rec = small.tile([128, B * H], F32, tag="r")
                nc.vector.reciprocal(rec[:], dsb[:])
                xo = sb.tile([128, B * C], F32, tag=f"xo{layer}")
                nc.vector.tensor_tensor(
                    out=xo[:].rearrange("p (x d) -> p x d", d=D),
                    in0=out_p[:].rearrange("p (x d) -> p x d", d=D),
                    in1=rec[:].rearrange("p (x u) -> p x u", u=1)
                        .to_broadcast([128, B * H, D]),
                    op=mybir.AluOpType.mult)
                return xo, den_p

            if stop_after not in ('A', 'B'):
              x1, _ = edge_loop(table1, 0)
              if stop_after == 'C1':
                nc.sync.dma_start(out=y_out[:], in_=x1[:])
            if stop_after not in ('A', 'B', 'C1'):
              if debug:
                nc.sync.dma_start(out=dbg["dbg_x1"][:], in_=x1[:])
                nc.sync.dma_start(out=dbg["dbg_tbl"][:], in_=table1[:])

              # ---- transpose x1 -> x1T [c-part, c-half, (n b)]
              x1T = sb.tile([128, 2, NB_LOCAL], F32)
              for b in range(B):
                for ch in range(2):
                    pt = ps_t.tile([128, 128], F32, space="PSUM", tag="pt", name="pt")
                    nc.tensor.transpose(
                        out=pt[:],
                        in_=x1[:, b * C + ch * 128: b * C + (ch + 1) * 128],
                        identity=ident[:])
                    nc.scalar.copy(
                        out=x1T[:, ch, :].rearrange(
                            "p (n b2) -> p n b2", b2=B)[:, :, b],
                        in_=pt[:])

              table2 = build_table(x1T, w_sb["wn2cols"], a2_sb, 2)
              if stop_after == 'D':
                dummy2 = sb.tile([128, B * C], F32)
                nc.sync.dma_start(out=dummy2[:], in_=table2[0:128, 0:B * C])
                nc.sync.dma_start(out=y_out[:], in_=dummy2[:])
              else:
                x2, den2 = edge_loop(table2, 1)
                if debug:
                    dsb2 = small.tile([128, B * H], F32, tag="dd")
                    nc.vector.tensor_copy(out=dsb2[:], in_=den2[:])
                    nc.sync.dma_start(out=dbg["dbg_den"][:], in_=dsb2[:])
                nc.sync.dma_start(out=y_out[:], in_=x2[:])

    nc.compile()
    return nc


class _StopBuild(Exception):
    def __init__(self, nc):
        self.nc = nc


_CACHE: dict = {}


def _get_program(B_pad: int, debug: bool = False):
    key = (B_pad, debug)
    if key not in _CACHE:
        _CACHE[key] = _build(B_pad, debug)
    return _CACHE[key]


def kernel(debug=False, trace=False, **inputs):
    in_maps, B_pad, E_pad, n_chunks = _prep(**inputs)
    nc = _get_program(B_pad, debug)
    res = run_bass_kernel_spmd(nc, in_maps, core_ids=list(range(NC)),
                               trace=trace)
    y = np.concatenate([res.results[k]["y"] for k in range(NC)], axis=0)
    out = y.reshape(N, B, C)
    if debug or trace:
        return out, res
    return out
